# revision 1
# baseline (speedup 1.0000x reference)
"""Trainium2 Bass kernel for HandDecoder-style GNN message passing.

Math (per batch element b):
  f = relu(MLP3([feature, coords]))                        # [N, C1=32]
  t1[i,j,h] = relu(a[j,h] + kb1[h] - a[i,h]),  a = coords @ kw1    # [N,N,8]
  t2[i,j,k] = relu(sum_h t1[i,j,h] kw2[h,k] + kb2[k])             # [N,N,16]
  g[j,k,d]  = sum_c f[j,c] kw3[k, c*16+d]                          # [N,16,16]
  out[i,d]  = relu(sum_{j,k} t2[i,j,k] g[j,k,d] + sum_c F[c] kb3[c*16+d])
  (F[c] = sum_j f[j,c])
This is algebraically identical to the reference (which materializes the
[N,N,C1*C2] pairwise kernel tensor) but ~15x less compute and no giant
intermediate. Data-parallel over batch: 4 batch elements per core, 8 cores.
"""

import sys
import numpy as np

for _p in ("/opt/trn_rl_repo",):
    if _p not in sys.path:
        sys.path.insert(0, _p)

import concourse.bass as bass
import concourse.bacc as bacc
import concourse.mybir as mybir
import concourse.tile as tile
from concourse.bass_utils import run_bass_kernel_spmd

B, N = 32, 128
C0, C1, C2 = 64, 32, 16
NCORES = 8
BPC = B // NCORES          # batches per core = 4
F32 = mybir.dt.float32
RELU = mybir.ActivationFunctionType.Relu
COPY = mybir.ActivationFunctionType.Copy

_CACHED_NC = None


def build_nc(stage=5):
    import os
    stage = int(os.environ.get("KSTAGE", stage))
    nc = bacc.Bacc("TRN2", target_bir_lowering=False, debug=False,
                   num_devices=NCORES)

    xT = nc.dram_tensor("xT", [67, BPC * N], F32, kind="ExternalInput").ap()
    c4T = nc.dram_tensor("c4T", [4, BPC * N], F32, kind="ExternalInput").ap()
    sel = nc.dram_tensor("sel", [8, BPC * 8 * 128], F32, kind="ExternalInput").ap()
    wb1 = nc.dram_tensor("wb1", [67, 100], F32, kind="ExternalInput").ap()
    wb2 = nc.dram_tensor("wb2", [128, 401], F32, kind="ExternalInput").ap()
    one512 = nc.dram_tensor("one512", [1, BPC * N], F32, kind="ExternalInput").ap()
    out_d = nc.dram_tensor("out", [N, BPC, C2], F32, kind="ExternalOutput").ap()
    g_dram = nc.dram_tensor("gscr", [BPC * 32768], F32).ap()
    a_dram = nc.dram_tensor("ascr", [4096], F32).ap()

    with tile.TileContext(nc) as tc:
        with (
            tc.tile_pool(name="const", bufs=1) as cpool,
            tc.tile_pool(name="work", bufs=1) as wpool,
            tc.tile_pool(name="ps_misc", bufs=2, space=bass.MemorySpace.PSUM) as pmisc,
            tc.tile_pool(name="ps_t1", bufs=2, space=bass.MemorySpace.PSUM) as pt1,
            tc.tile_pool(name="ps_t2", bufs=2, space=bass.MemorySpace.PSUM) as pt2,
        ):
            # ---- load inputs ----
            xT_s = cpool.tile([67, BPC * N], F32, tag="xT")
            c4T_s = cpool.tile([4, BPC * N], F32, tag="c4T")
            wb1_s = cpool.tile([67, 100], F32, tag="wb1")
            wb2_s = cpool.tile([128, 401], F32, tag="wb2")
            lhsT_all = cpool.tile([9, BPC * 8 * 128], F32, tag="lhsT")
            rhs9 = cpool.tile([9, BPC * N], F32, tag="rhs9")
            nc.sync.dma_start(xT_s[:], xT)
            nc.sync.dma_start(c4T_s[:], c4T)
            nc.sync.dma_start(lhsT_all[0:8, :], sel)
            nc.sync.dma_start(wb1_s[:], wb1)
            nc.sync.dma_start(wb2_s[:], wb2)
            nc.sync.dma_start(rhs9[8:9, :], one512)
            ones128 = cpool.tile([1, 128], F32, tag="ones")
            nc.vector.memset(ones128[:], 1.0)

            # weight-blob slices
            dw1 = wb1_s[0:67, 0:32]
            dw2 = wb1_s[0:32, 32:48]
            dw3 = wb1_s[0:16, 48:80]
            kw1n4 = wb1_s[0:4, 80:88]     # [-kw1; 0]
            kw1b4 = wb1_s[0:4, 88:96]     # [kw1; kb1]
            db1 = wb1_s[0:32, 96:97]
            db2 = wb1_s[0:16, 97:98]
            db3 = wb1_s[0:32, 98:99]
            kb2t = wb2_s[0:128, 0:1]
            kw2bd = wb2_s[0:128, 1:129]   # block-diag kw2, duplicated halves
            kw3p = wb2_s[0:32, 129:385]
            kb3r = wb2_s[0:32, 385:401]

            # ---- a-stage: a_neg = -(coords @ kw1), transposed [8,(b,i)] ----
            aneg_ps = pmisc.tile([8, BPC * N], F32, tag="m")
            nc.tensor.matmul(aneg_ps[:], kw1n4, c4T_s[:])
            nc.scalar.activation(rhs9[0:8, :], aneg_ps[:], COPY)

            # ---- a2b = coords @ kw1 + kb1, row-major [j, (b,h)] ----
            a2b_ps = pmisc.tile([128, BPC * 8], F32, tag="m")
            for b in range(BPC):
                nc.tensor.matmul(a2b_ps[:, b * 8:(b + 1) * 8],
                                 c4T_s[0:4, b * N:(b + 1) * N], kw1b4)
            a2b_sb = wpool.tile([128, BPC * 8], F32, tag="a2b")
            nc.scalar.activation(a2b_sb[:], a2b_ps[:], COPY)
            # scatter a2b into row 8 of lhsT_all via a DRAM bounce
            # (keeps every SBUF-side DMA AP partition-leading):
            # ascr[(jc*16+jl)*32 + b*8 + h] = a2b_sb[jc*16+jl, b*8+h]  (row-major)
            # lhsT_all[8, b*1024 + jc*128 + jl*8 + h] <- ascr 3-dim gather
            nc.sync.dma_start(a_dram, a2b_sb[:])
            src = a_dram.rearrange("(j b h) -> b j h", j=128, h=8)
            nc.sync.dma_start(lhsT_all[8:9, :], src)

            # ---- decode MLP: fT [32, (b,n)] ----
            h1 = wpool.tile([32, BPC * N], F32, tag="h1")
            h2 = wpool.tile([16, BPC * N], F32, tag="h2")
            fT = wpool.tile([32, BPC * N], F32, tag="fT")
            d1_ps = pmisc.tile([32, BPC * N], F32, tag="m")
            nc.tensor.matmul(d1_ps[:], dw1, xT_s[:])
            nc.scalar.activation(h1[:], d1_ps[:], RELU, bias=db1)
            d2_ps = pmisc.tile([16, BPC * N], F32, tag="m")
            nc.tensor.matmul(d2_ps[:], dw2, h1[:])
            nc.scalar.activation(h2[:], d2_ps[:], RELU, bias=db2)
            d3_ps = pmisc.tile([32, BPC * N], F32, tag="m")
            nc.tensor.matmul(d3_ps[:], dw3, h2[:])
            nc.scalar.activation(fT[:], d3_ps[:], RELU, bias=db3)

            if stage == 1:
                dbg = wpool.tile([128, BPC * C2], F32, tag="dbg")
                nc.vector.memset(dbg[:], 0.0)
                nc.vector.tensor_scalar(dbg[0:32, 0:64], fT[:, 0:64], 0.0, None,
                                        mybir.AluOpType.add)
                nc.sync.dma_start(out_d, dbg[:])
            if stage >= 2:
                # ---- t1 stage: 64 matmuls [9,64]x[9,128] -> relu ----
                # tile idx2=(b*8+jc)*2+half: partitions p=jl*8+h (jl within half),
                # free i. All tiles are 64-partition, base 0 (HW dislikes base-64
                # PE operands).
                t1_sb = []
                t1_ps_tiles = []
                for grp in range(16):         # 4 chunk-halves per group
                    ps = pt1.tile([64, 512], F32, tag="t1ps")
                    t1_ps_tiles.append(ps)
                    sb = wpool.tile([64, 512], F32, tag=f"t1sb{grp}")
                    t1_sb.append(sb)
                for b in range(BPC):
                    for jc in range(8):
                        for half in range(2):
                            idx2 = (b * 8 + jc) * 2 + half
                            grp, q = divmod(idx2, 4)
                            col = (b * 8 + jc) * 128 + half * 64
                            nc.tensor.matmul(
                                t1_ps_tiles[grp][:, q * 128:(q + 1) * 128],
                                lhsT_all[0:9, col:col + 64],
                                rhs9[0:9, b * N:(b + 1) * N])
                for grp in range(16):
                    if grp % 2 == 0:
                        nc.scalar.activation(t1_sb[grp][:], t1_ps_tiles[grp][:], RELU)
                    else:
                        nc.vector.tensor_scalar(
                            t1_sb[grp][:], t1_ps_tiles[grp][:], 0.0, None,
                            mybir.AluOpType.max)

                if stage == 2:
                    dbg = wpool.tile([128, BPC * C2], F32, tag="dbg")
                    nc.scalar.activation(dbg[0:64, :], t1_sb[15][:, 0:BPC * C2], COPY)
                    nc.scalar.activation(dbg[64:128, :], t1_sb[14][:, 0:BPC * C2], COPY)
                    nc.sync.dma_start(out_d, dbg[:])
            if stage >= 3:
                # ---- t2 stage: 64 matmuls lhsT=kw2bd [64,128] ----
                # chunk c2 = jc*2 + half covers j in [c2*8, c2*8+8);
                # output partition p = jl*16 + k.
                t2_sb = []
                t2_ps_tiles = []
                for grp in range(16):
                    ps = pt2.tile([128, 512], F32, tag="t2ps")
                    t2_ps_tiles.append(ps)
                    sb = wpool.tile([128, 512], F32, tag=f"t2sb{grp}")
                    t2_sb.append(sb)
                kw2bd0 = wb2_s[0:64, 1:129]
                for b in range(BPC):
                    for jc in range(8):
                        for half in range(2):
                            idx = (b * 8 + jc) * 2 + half     # 0..63
                            grp, q = divmod(idx, 4)
                            nc.tensor.matmul(
                                t2_ps_tiles[grp][:, q * 128:(q + 1) * 128],
                                kw2bd0,
                                t1_sb[grp][:, q * 128:(q + 1) * 128])
                for grp in range(16):
                    if grp % 2 == 0:
                        nc.scalar.activation(t2_sb[grp][:], t2_ps_tiles[grp][:],
                                             RELU, bias=kb2t)
                    else:
                        nc.vector.tensor_scalar(
                            t2_sb[grp][:], t2_ps_tiles[grp][:], kb2t, 0.0,
                            mybir.AluOpType.add, mybir.AluOpType.max)

            if stage == 3:
                dbg = wpool.tile([128, BPC * C2], F32, tag="dbg")
                nc.scalar.activation(dbg[:], t2_sb[15][:, 0:BPC * C2], COPY)
                nc.sync.dma_start(out_d, dbg[:])
            if stage >= 4:
                # ---- g stage (+ bias2) ----
                with tc.tile_pool(name="ps_g", bufs=1,
                                  space=bass.MemorySpace.PSUM) as pg:
                    g_ps = pg.tile([128, BPC * 256], F32, tag="g")
                    g_rm = wpool.tile([128, BPC * 256], F32, tag="grm")
                    g_all = wpool.tile([128, BPC * 256], F32, tag="gall")
                    for b in range(BPC):
                        nc.tensor.matmul(g_ps[:, b * 256:(b + 1) * 256],
                                         fT[0:32, b * N:(b + 1) * N], kw3p)
                    for b in range(BPC):
                        if b % 2 == 0:
                            nc.scalar.activation(g_rm[:, b * 256:(b + 1) * 256],
                                                 g_ps[:, b * 256:(b + 1) * 256], COPY)
                        else:
                            nc.vector.tensor_copy(g_rm[:, b * 256:(b + 1) * 256],
                                                  g_ps[:, b * 256:(b + 1) * 256])
                    # reshape: g_all[(jl k), b*256 + c*16 + d] = g_rm[c*8+jl, b*256+k*16+d]
                    # bounce through DRAM: SBUF partition-regroup DMAs are limited
                    # (partition dim must lead / span 128), DRAM APs are linear.
                    # A) g_dram[b*32768 + j*256 + k*16 + d] = g_rm row-major
                    # B) g_all[(jl*16+k), b*256+c2*16+d] <- gather (jlk, c2, d) 3-dim AP
                    for b in range(BPC):
                        eng = nc.sync if b % 2 == 0 else nc.scalar
                        eng.dma_start(g_dram[b * 32768:(b + 1) * 32768],
                                      g_rm[:, b * 256:(b + 1) * 256])
                    for b in range(BPC):
                        eng = nc.sync if b % 2 == 0 else nc.scalar
                        dstB = g_all[:, b * 256:(b + 1) * 256].rearrange(
                            "p (c d) -> p c d", d=16)
                        srcB = g_dram[b * 32768:(b + 1) * 32768].rearrange(
                            "(c jk d) -> jk c d", jk=128, d=16)
                        eng.dma_start(dstB, srcB)

                    # bias2[b,d] = sum_c F[b,c] kb3r[c,d];  F = rowsum of f
                    F_sb = wpool.tile([32, BPC], F32, tag="F")
                    for b in range(BPC):
                        nc.vector.tensor_reduce(F_sb[:, b:b + 1],
                                                fT[0:32, b * N:(b + 1) * N],
                                                mybir.AxisListType.X,
                                                mybir.AluOpType.add)
                    bias2_ps = pmisc.tile([1, BPC * C2], F32, tag="m")
                    for b in range(BPC):
                        nc.tensor.matmul(bias2_ps[0:1, b * C2:(b + 1) * C2],
                                         F_sb[0:32, b:b + 1], kb3r)
                    bias2_sb = wpool.tile([1, BPC * C2], F32, tag="b2")
                    nc.scalar.activation(bias2_sb[:], bias2_ps[:], COPY)

            if stage == 4:
                dbg = wpool.tile([128, BPC * C2], F32, tag="dbg")
                nc.scalar.activation(dbg[:], g_all[:, 0:BPC * C2], COPY)
                nc.sync.dma_start(out_d, dbg[:])
            if stage >= 5:
                # ---- final contraction ----
                with tc.tile_pool(name="ps_out", bufs=1,
                                  space=bass.MemorySpace.PSUM) as pout:
                    out_ps = pout.tile([128, BPC * C2], F32, tag="o")
                    for b in range(BPC):
                        for c2 in range(16):
                            idx = (b * 8 + c2 // 2) * 2 + c2 % 2
                            grp, q = divmod(idx, 4)
                            nc.tensor.matmul(
                                out_ps[:, b * C2:(b + 1) * C2],
                                t2_sb[grp][:, q * 128:(q + 1) * 128],
                                g_all[:, b * 256 + c2 * 16: b * 256 + (c2 + 1) * 16],
                                start=(c2 == 0), stop=False)
                        nc.tensor.matmul(out_ps[:, b * C2:(b + 1) * C2],
                                         ones128[0:1, 0:128],
                                         bias2_sb[0:1, b * C2:(b + 1) * C2],
                                         start=False, stop=True)
                    out_sb = wpool.tile([128, BPC * C2], F32, tag="osb")
                    nc.scalar.activation(out_sb[:], out_ps[:], RELU)
                    nc.sync.dma_start(out_d, out_sb[:])

    nc.compile()
    return nc


def _host_inputs(feature, coordinates_v, dw1, db1, dw2, db2, dw3, db3,
                 kw1, kb1, kw2, kb2, kw3, kb3):
    """Per-core input maps. Pure layout transforms, no FLOPs."""
    f32 = np.float32
    # wb1: small weights packed column-wise into a [67, 100] blob
    wb1 = np.zeros((67, 100), f32)
    wb1[0:67, 0:32] = dw1
    wb1[0:32, 32:48] = dw2
    wb1[0:16, 48:80] = dw3
    wb1[0:3, 80:88] = -kw1
    wb1[0:3, 88:96] = kw1
    wb1[3, 88:96] = kb1
    wb1[0:32, 96] = db1
    wb1[0:16, 97] = db2
    wb1[0:32, 98] = db3
    # wb2: kb2 tiled, block-diag kw2 (dup halves), permuted kw3, kb3
    wb2 = np.zeros((128, 401), f32)
    wb2[:, 0] = np.tile(kb2, 8)
    bd = np.zeros((64, 128), f32)
    for jl in range(8):
        bd[jl * 8:(jl + 1) * 8, jl * 16:(jl + 1) * 16] = kw2
    wb2[0:64, 1:129] = bd
    wb2[64:128, 1:129] = bd
    wb2[0:32, 129:385] = kw3.reshape(16, 32, 16).transpose(1, 0, 2).reshape(32, 256)
    wb2[0:32, 385:401] = kb3.reshape(32, 16)
    # selector rows for the t1 matmul
    cols = np.arange(BPC * 8 * 128)
    sel = (cols[None, :] % 8 == np.arange(8)[:, None]).astype(f32)

    in_maps = []
    for c in range(NCORES):
        fe = feature[c * BPC:(c + 1) * BPC]          # [4, 64]
        co = coordinates_v[c * BPC:(c + 1) * BPC]    # [4, 128, 3]
        xT = np.empty((67, BPC * N), f32)
        c4T = np.empty((4, BPC * N), f32)
        for b in range(BPC):
            xT[0:64, b * N:(b + 1) * N] = fe[b][:, None]
            xT[64:67, b * N:(b + 1) * N] = co[b].T
            c4T[0:3, b * N:(b + 1) * N] = co[b].T
        c4T[3, :] = 1.0
        in_maps.append({"xT": np.ascontiguousarray(xT),
                        "c4T": np.ascontiguousarray(c4T),
                        "sel": sel, "wb1": wb1, "wb2": wb2,
                        "one512": np.ones((1, BPC * N), f32)})
    return in_maps


def kernel(**inputs):
    global _CACHED_NC
    if _CACHED_NC is None:
        _CACHED_NC = build_nc()
    nc = _CACHED_NC
    in_maps = _host_inputs(
        np.asarray(inputs["feature"]), np.asarray(inputs["coordinates_v"]),
        np.asarray(inputs["dw1"]), np.asarray(inputs["db1"]),
        np.asarray(inputs["dw2"]), np.asarray(inputs["db2"]),
        np.asarray(inputs["dw3"]), np.asarray(inputs["db3"]),
        np.asarray(inputs["kw1"]), np.asarray(inputs["kb1"]),
        np.asarray(inputs["kw2"]), np.asarray(inputs["kb2"]),
        np.asarray(inputs["kw3"]), np.asarray(inputs["kb3"]))
    res = run_bass_kernel_spmd(nc, in_maps, list(range(NCORES)))
    out = np.empty((B, N, C2), np.float32)
    for c in range(NCORES):
        # per-core out is [N(i), BPC(b), C2(d)]
        out[c * BPC:(c + 1) * BPC] = res.results[c]["out"].transpose(1, 0, 2)
    return out



# revision 9
# speedup vs baseline: 2.7394x; 2.7394x over previous
"""Trainium2 Bass kernel for HandDecoder-style GNN message passing.

Math (per batch element b):
  f = relu(MLP3([feature, coords]))                        # [N, C1=32]
  t1[i,j,h] = relu(a[j,h] + kb1[h] - a[i,h]),  a = coords @ kw1    # [N,N,8]
  t2[i,j,k] = relu(sum_h t1[i,j,h] kw2[h,k] + kb2[k])             # [N,N,16]
  g[j,k,d]  = sum_c f[j,c] kw3[k, c*16+d]                          # [N,16,16]
  out[i,d]  = relu(sum_{j,k} t2[i,j,k] g[j,k,d] + sum_c F[c] kb3[c*16+d])
  (F[c] = sum_j f[j,c])

v3 layout (all matmuls bf16, fp32 PSUM):
  - t1 [(jl16,h8)=128p, (b,i)=512f] per j-chunk of 16: ONE aneg matmul;
    bias+relu as 32 per-partition-scalar ops spread over DVE (SBUF 2x,
    from a DMA-evacuated aneg copy), GpSimd (SBUF), and ScalarE (PSUM).
  - t2 [(jl8,k16)=128p, (b,i)=512f] per j-chunk of 8: 16 matmuls (N=512)
    into paired 2-bank PSUM tiles, evacuated as 8 [128,1024] relu ops.
  - g: natural [j,(k,d)] matmuls; DRAM bounce into [(jl,k), (T,d)].
  - final: 64 col-tiled (128x32 mode) matmuls, 4 concurrent PSUM
    quadrants (quadrant b rows 32b..32b+16 hold out[d,i] for batch b);
    bias2 folded into the output relu as a per-partition bias.
Data-parallel over batch: 4 batch elements per core, 8 cores.
"""

import sys
import numpy as np
import ml_dtypes

for _p in ("/opt/trn_rl_repo",):
    if _p not in sys.path:
        sys.path.insert(0, _p)

import concourse.bass as bass
import concourse.bacc as bacc
import concourse.mybir as mybir
import concourse.tile as tile
from concourse.bass_utils import run_bass_kernel_spmd

B, N = 32, 128
C0, C1, C2 = 64, 32, 16
NCORES = 8
BPC = B // NCORES          # batches per core = 4
F32 = mybir.dt.float32
BF16 = mybir.dt.bfloat16
RELU = mybir.ActivationFunctionType.Relu
COPY = mybir.ActivationFunctionType.Copy
ADD = mybir.AluOpType.add
MAX = mybir.AluOpType.max
BF = ml_dtypes.bfloat16

_CACHED_NC = None


def build_nc():
    nc = bacc.Bacc("TRN2", target_bir_lowering=False, debug=False,
                   num_devices=NCORES)

    c4T_d = nc.dram_tensor("c4T", [4, BPC * N], BF16, kind="ExternalInput").ap()
    c4X_d = nc.dram_tensor("c4X", [49, 32], BF16, kind="ExternalInput").ap()
    xT_d = nc.dram_tensor("xT", [67, BPC * N], BF16, kind="ExternalInput").ap()
    wbC_d = nc.dram_tensor("wbC", [49, 256], BF16, kind="ExternalInput").ap()
    wbB_d = nc.dram_tensor("wbB", [67, 80], BF16, kind="ExternalInput").ap()
    kw2AB_d = nc.dram_tensor("kw2AB", [128, 256], BF16, kind="ExternalInput").ap()
    kw3g_d = nc.dram_tensor("kw3g", [32, 256], BF16, kind="ExternalInput").ap()
    kb3q_d = nc.dram_tensor("kb3q", [32, 512], BF16, kind="ExternalInput").ap()
    fbias_d = nc.dram_tensor("fbias", [128, 4], F32, kind="ExternalInput").ap()
    out_d = nc.dram_tensor("out", [128, N], F32, kind="ExternalOutput").ap()
    g_dram = nc.dram_tensor("gscr", [BPC * 32768], BF16).ap()

    with tile.TileContext(nc) as tc:
        with (
            tc.tile_pool(name="const", bufs=1) as cpool,
            tc.tile_pool(name="work", bufs=1) as wpool,
            tc.tile_pool(name="ps_aneg", bufs=1, space=bass.MemorySpace.PSUM) as pan,
            tc.tile_pool(name="ps_misc", bufs=1, space=bass.MemorySpace.PSUM) as pmisc,
            tc.tile_pool(name="ps_g", bufs=1, space=bass.MemorySpace.PSUM) as pg,
            tc.tile_pool(name="ps_t2", bufs=2, space=bass.MemorySpace.PSUM) as pt2,
            tc.tile_pool(name="ps_out", bufs=1, space=bass.MemorySpace.PSUM) as pout,
        ):
            # ---- input loads ----
            c4T = cpool.tile([4, BPC * N], BF16, tag="c4T")
            c4X = cpool.tile([49, 32], BF16, tag="c4X")
            xT = cpool.tile([67, BPC * N], BF16, tag="xT")
            wbC = cpool.tile([49, 256], BF16, tag="wbC")
            wbB = cpool.tile([67, 80], BF16, tag="wbB")
            kw2AB = cpool.tile([128, 256], BF16, tag="kw2AB")
            kw3g = cpool.tile([32, 256], BF16, tag="kw3g")
            kb3q = cpool.tile([32, 512], BF16, tag="kb3q")
            fbias = cpool.tile([128, 4], F32, tag="fbias")
            nc.sync.dma_start(c4X[:], c4X_d)
            nc.sync.dma_start(c4T[:], c4T_d)
            nc.sync.dma_start(wbC[:], wbC_d)
            nc.sync.dma_start(xT[:], xT_d)
            nc.sync.dma_start(wbB[:], wbB_d)
            nc.sync.dma_start(fbias[:], fbias_d)
            nc.sync.dma_start(kw2AB[:], kw2AB_d)
            nc.sync.dma_start(kw3g[:], kw3g_d)
            nc.sync.dma_start(kb3q[:], kb3q_d)

            L49 = wbC[0:49, 0:128]
            kw1negr = wbC[0:4, 128:256]
            dw1 = wbB[0:67, 0:32]
            dw2 = wbB[0:32, 32:48]
            dw3 = wbB[0:16, 48:80]
            kw3p = kw3g[0:32, 0:256]
            db1 = fbias[0:32, 0:1]
            db2 = fbias[0:16, 1:2]
            db3 = fbias[0:32, 2:3]
            kb2t = fbias[0:128, 3:4]

            # ---- M2: a2b[(jl,h), (b,chunk)] = a[chunk*16+jl, h] + kb1[h]
            a2b_ps = pmisc.tile([128, 32], F32, tag="m")
            nc.tensor.matmul(a2b_ps[:], L49, c4X[:])
            # ---- M1: aneg_ps[(jl,h), (b,i)] = -a[i,h]
            aneg_ps = pan.tile([128, BPC * N], F32, tag="aneg")
            nc.tensor.matmul(aneg_ps[:], kw1negr, c4T[:])
            a2b = wpool.tile([128, 32], F32, tag="a2b")
            nc.scalar.activation(a2b[:], a2b_ps[:], COPY)
            # Evacuate aneg so DVE (2x SBUF mode) / GpSimd can read it
            aneg_sb = wpool.tile([128, BPC * N], F32, tag="anegsb")
            nc.scalar.activation(aneg_sb[:], aneg_ps[:], COPY)

            # ---- decode MLP -> fT [32, (b,i)] bf16 ----
            h1 = wpool.tile([32, BPC * N], BF16, tag="h1")
            h2 = wpool.tile([16, BPC * N], BF16, tag="h2")
            fT = wpool.tile([32, BPC * N], BF16, tag="fT")
            d1_ps = pmisc.tile([32, BPC * N], F32, tag="m")
            nc.tensor.matmul(d1_ps[:], dw1, xT[:])
            nc.scalar.activation(h1[:], d1_ps[:], RELU, bias=db1)
            d2_ps = pmisc.tile([16, BPC * N], F32, tag="m")
            nc.tensor.matmul(d2_ps[:], dw2, h1[:])
            nc.scalar.activation(h2[:], d2_ps[:], RELU, bias=db2)
            d3_ps = pmisc.tile([32, BPC * N], F32, tag="m")
            nc.tensor.matmul(d3_ps[:], dw3, h2[:])
            nc.scalar.activation(fT[:], d3_ps[:], RELU, bias=db3)

            # ---- t1: relu(aneg + a2b-bias), 32 ops over 3 engines ----
            # DVE x16 (SBUF 2x), GpSimd x8 (SBUF), ScalarE x8 (PSUM)
            t1_sb = [wpool.tile([128, BPC * N], BF16, tag=f"t1_{c}",
                                name=f"t1sb{c}") for c in range(8)]
            for c in range(8):
                for b in range(BPC):
                    idx = c * BPC + b
                    out_ap = t1_sb[c][:, b * N:(b + 1) * N]
                    bias_ap = a2b[:, b * 8 + c:b * 8 + c + 1]
                    sb_in = aneg_sb[:, b * N:(b + 1) * N]
                    ps_in = aneg_ps[:, b * N:(b + 1) * N]
                    if idx % 4 in (0, 2):
                        nc.vector.tensor_scalar(out_ap, sb_in, bias_ap, 0.0,
                                                ADD, MAX)
                    elif idx % 4 == 1:
                        nc.gpsimd.tensor_scalar(out_ap, sb_in, bias_ap, 0.0,
                                                ADD, MAX)
                    else:
                        nc.scalar.activation(out_ap, ps_in, RELU, bias=bias_ap)

            # ---- g: g_rm[j, (b,k,d)] -> DRAM bounce -> g_all[(jl,k),(b,T,d)]
            g_rm = wpool.tile([128, BPC * 256], BF16, tag="grm")
            g_all = wpool.tile([128, BPC * 256], BF16, tag="gall")
            for half in range(2):
                g_ps = pg.tile([128, 512], F32, tag="g", name=f"gps{half}")
                for bb in range(2):
                    b = half * 2 + bb
                    nc.tensor.matmul(g_ps[:, bb * 256:(bb + 1) * 256],
                                     fT[0:32, b * N:(b + 1) * N], kw3p)
                dst = g_rm[:, half * 512:(half + 1) * 512]
                if half == 0:
                    nc.vector.tensor_copy(dst, g_ps[:])
                else:
                    nc.scalar.activation(dst, g_ps[:], COPY)
                for bb in range(2):
                    b = half * 2 + bb
                    qe = nc.sync if bb == 0 else nc.scalar
                    qe.dma_start(g_dram[b * 32768:(b + 1) * 32768],
                                 g_rm[:, b * 256:(b + 1) * 256])
            for b in range(BPC):
                qe = nc.sync if b % 2 == 0 else nc.scalar
                dstB = g_all[:, b * 256:(b + 1) * 256].rearrange(
                    "p (c d) -> p c d", d=16)
                srcB = g_dram[b * 32768:(b + 1) * 32768].rearrange(
                    "(c jk d) -> jk c d", jk=128, d=16)
                qe.dma_start(dstB, srcB)

            # ---- bias2 in quadrant layout: [32b+d, 1] ----
            F_f32 = wpool.tile([32, BPC], F32, tag="Ff")
            F_sb = wpool.tile([32, BPC], BF16, tag="F")
            for b in range(BPC):
                nc.vector.tensor_reduce(F_f32[:, b:b + 1],
                                        fT[0:32, b * N:(b + 1) * N],
                                        mybir.AxisListType.X, ADD)
            nc.gpsimd.tensor_copy(F_sb[:], F_f32[:])
            b2_ps = pmisc.tile([128, 1], F32, tag="m")
            for b in range(BPC):
                nc.tensor.matmul(b2_ps[:], kb3q[:, b * 128:(b + 1) * 128],
                                 F_sb[0:32, b:b + 1],
                                 start=(b == 0), stop=(b == 3))
            b2q = wpool.tile([128, 1], F32, tag="b2q")
            nc.scalar.activation(b2q[:], b2_ps[:], COPY)

            # ---- t2: 16 matmuls N=512 into paired 2-bank PSUM tiles ----
            t2_sb = [wpool.tile([128, 2 * BPC * N], BF16, tag=f"t2_{p}",
                                name=f"t2sb{p}") for p in range(8)]
            for p in range(8):
                t2_ps = pt2.tile([128, 1024], F32, tag="t2ps",
                                 name=f"t2ps{p}")
                for s in range(2):
                    t = 2 * p + s
                    c, half = divmod(t, 2)
                    nc.tensor.matmul(t2_ps[:, s * 512:(s + 1) * 512],
                                     kw2AB[:, half * 128:(half + 1) * 128],
                                     t1_sb[c][:])
                if p % 2 == 0:
                    nc.vector.tensor_scalar(t2_sb[p][:], t2_ps[:], kb2t, 0.0,
                                            ADD, MAX)
                else:
                    nc.scalar.activation(t2_sb[p][:], t2_ps[:], RELU,
                                         bias=kb2t)

            # ---- final: 64 col-tiled matmuls, PSUM quadrants ----
            out_ps = pout.tile([128, N], F32, tag="o")
            for t in range(16):
                p, s = divmod(t, 2)
                for b in range(BPC):
                    nc.tensor.matmul(
                        out_ps[32 * b:32 * b + C2, :],
                        g_all[:, b * 256 + t * 16:b * 256 + (t + 1) * 16],
                        t2_sb[p][:, s * 512 + b * N:s * 512 + (b + 1) * N],
                        start=(t == 0), stop=(t == 15),
                        tile_position=(0, 32 * b))
            out_sb = wpool.tile([128, N], F32, tag="osb")
            nc.scalar.activation(out_sb[:], out_ps[:], RELU, bias=b2q[:])
            nc.sync.dma_start(out_d, out_sb[:])

    nc.compile()
    return nc


def _host_inputs(feature, coordinates_v, dw1, db1, dw2, db2, dw3, db3,
                 kw1, kb1, kw2, kb2, kw3, kb3):
    """Per-core input maps. Pure layout transforms, no FLOPs."""
    f32 = np.float32
    # wbC: [49,0:128] = L49 (jl-selector x kw1, kb1 row), [0:4,128:256] = -kw1 rep
    wbC = np.zeros((49, 256), f32)
    for jl in range(16):
        for c in range(3):
            wbC[c * 16 + jl, jl * 8:(jl + 1) * 8] = kw1[c]
    wbC[48, 0:128] = np.tile(kb1, 16)
    wbC[0:3, 128:256] = np.tile((-kw1)[:, None, :], (1, 16, 1)).reshape(3, 128)

    wbB = np.zeros((67, 80), f32)
    wbB[0:67, 0:32] = dw1
    wbB[0:32, 32:48] = dw2
    wbB[0:16, 48:80] = dw3

    # kw2AB: two block-diag stationaries [(jl16,h8) x (jl8,k16)]
    kw2AB = np.zeros((128, 256), f32)
    for jl8 in range(8):
        kw2AB[jl8 * 8:(jl8 + 1) * 8, jl8 * 16:(jl8 + 1) * 16] = kw2
        kw2AB[(jl8 + 8) * 8:(jl8 + 9) * 8, 128 + jl8 * 16:128 + (jl8 + 1) * 16] = kw2

    # kw3p: [c, (k,d)]
    kw3g = kw3.reshape(16, 32, 16).transpose(1, 0, 2).reshape(32, 256).astype(f32)

    # kb3q: 4 shifted copies of kb3r [32, 16] -> cols 32b..32b+16
    kb3r = kb3.reshape(32, 16)
    kb3q = np.zeros((32, 512), f32)
    for b in range(4):
        kb3q[:, b * 128 + 32 * b:b * 128 + 32 * b + 16] = kb3r

    fbias = np.zeros((128, 4), f32)
    fbias[0:32, 0] = db1
    fbias[0:16, 1] = db2
    fbias[0:32, 2] = db3
    fbias[:, 3] = np.tile(kb2, 8)

    in_maps = []
    for cix in range(NCORES):
        fe = feature[cix * BPC:(cix + 1) * BPC]          # [4, 64]
        co = coordinates_v[cix * BPC:(cix + 1) * BPC]    # [4, 128, 3]
        xT = np.empty((67, BPC * N), f32)
        c4T = np.empty((4, BPC * N), f32)
        for b in range(BPC):
            xT[0:64, b * N:(b + 1) * N] = fe[b][:, None]
            xT[64:67, b * N:(b + 1) * N] = co[b].T
            c4T[0:3, b * N:(b + 1) * N] = co[b].T
        c4T[3, :] = 1.0
        # c4X[(c,jl'), (b,chunk)] = co[b, chunk*16+jl', c]; row 48 = 1
        c4X = np.empty((49, 32), f32)
        cr = co.transpose(2, 0, 1).reshape(3, BPC, 8, 16)  # [c, b, chunk, jl]
        c4X[0:48] = cr.transpose(0, 3, 1, 2).reshape(48, 32)
        c4X[48, :] = 1.0
        in_maps.append({
            "c4T": c4T.astype(BF), "c4X": c4X.astype(BF),
            "xT": xT.astype(BF), "wbC": wbC.astype(BF),
            "wbB": wbB.astype(BF), "kw2AB": kw2AB.astype(BF),
            "kw3g": kw3g.astype(BF), "kb3q": kb3q.astype(BF),
            "fbias": fbias})
    return in_maps


def kernel(**inputs):
    global _CACHED_NC
    if _CACHED_NC is None:
        _CACHED_NC = build_nc()
    nc = _CACHED_NC
    in_maps = _host_inputs(
        np.asarray(inputs["feature"]), np.asarray(inputs["coordinates_v"]),
        np.asarray(inputs["dw1"]), np.asarray(inputs["db1"]),
        np.asarray(inputs["dw2"]), np.asarray(inputs["db2"]),
        np.asarray(inputs["dw3"]), np.asarray(inputs["db3"]),
        np.asarray(inputs["kw1"]), np.asarray(inputs["kb1"]),
        np.asarray(inputs["kw2"]), np.asarray(inputs["kb2"]),
        np.asarray(inputs["kw3"]), np.asarray(inputs["kb3"]))
    res = run_bass_kernel_spmd(nc, in_maps, list(range(NCORES)))
    out = np.empty((B, N, C2), np.float32)
    for cix in range(NCORES):
        r = res.results[cix]["out"]      # [128, N] quadrants
        for b in range(BPC):
            out[cix * BPC + b] = r[32 * b:32 * b + C2, :].T
    return out


# revision 10
# speedup vs baseline: 3.0361x; 1.1083x over previous
"""Trainium2 Bass kernel for HandDecoder-style GNN message passing.

Math (per batch element b):
  f = relu(MLP3([feature, coords]))                        # [N, C1=32]
  t1[i,j,h] = relu(a[j,h] + kb1[h] - a[i,h]),  a = coords @ kw1    # [N,N,8]
  t2[i,j,k] = relu(sum_h t1[i,j,h] kw2[h,k] + kb2[k])             # [N,N,16]
  g[j,k,d]  = sum_c f[j,c] kw3[k, c*16+d]                          # [N,16,16]
  out[i,d]  = relu(sum_{j,k} t2[i,j,k] g[j,k,d] + sum_c F[c] kb3[c*16+d])
  (F[c] = sum_j f[j,c])

v4 (all matmuls bf16, fp32 PSUM):
  - t1 pre-activation computed entirely on the PE: per j-chunk of 16, a
    K=8 matmul with a composite stationary [rows 0-2: -kw1 replicated;
    rows 4-7: a2b values for the chunk's 4 batches] against an
    augmented rhs [coords; one-hot batch selector]. The a2b rows are
    produced by one matmul (a2bT [32,128]) and routed into the blob via
    a 4-descriptor DRAM bounce. Evacuation = plain relu copies.
  - t2: 16 matmuls (N=512) into paired 2-bank PSUM tiles; 8 [128,1024]
    relu(+kb2) evacuations alternating DVE/ScalarE.
  - final: 64 col-tiled (128x32) matmuls, 4 concurrent PSUM quadrants;
    bias2 folded into the output relu as a per-partition bias.
  - all constant/input tensors packed into 4 DMAs.
Data-parallel over batch: 4 batch elements per core, 8 cores.
"""

import sys
import numpy as np
import ml_dtypes

for _p in ("/opt/trn_rl_repo",):
    if _p not in sys.path:
        sys.path.insert(0, _p)

import concourse.bass as bass
import concourse.bacc as bacc
import concourse.mybir as mybir
import concourse.tile as tile
from concourse.bass_utils import run_bass_kernel_spmd

B, N = 32, 128
C0, C1, C2 = 64, 32, 16
NCORES = 8
BPC = B // NCORES          # batches per core = 4
F32 = mybir.dt.float32
BF16 = mybir.dt.bfloat16
RELU = mybir.ActivationFunctionType.Relu
COPY = mybir.ActivationFunctionType.Copy
ADD = mybir.AluOpType.add
MAX = mybir.AluOpType.max
BF = ml_dtypes.bfloat16

# blobA column map [49 rows, 1696 cols]
A_C8T = 0          # [0:8,   0:512]   coords rows 0-2, row3=1, rows 4-7 b-sel
A_C4X = 512        # [0:49,  512:544]
A_L49 = 544        # [0:49,  544:672]
A_COMP = 672       # [0:8,   672:1696] rows 0-3 host const, rows 4-7 bounced
# blobB column map [67 rows, 1360 cols]
B_XT = 0           # [0:67,  0:512]
B_DW1 = 512        # [0:67,  512:544]
B_DW2 = 544        # [0:32,  544:560]
B_DW3 = 560        # [0:16,  560:592]
B_KW3 = 592        # [0:32,  592:848]
B_KB3Q = 848       # [0:32,  848:1360]

_CACHED_NC = None


def build_nc():
    nc = bacc.Bacc("TRN2", target_bir_lowering=False, debug=False,
                   num_devices=NCORES)

    blobA_d = nc.dram_tensor("blobA", [49, 1696], BF16, kind="ExternalInput").ap()
    blobB_d = nc.dram_tensor("blobB", [67, 1360], BF16, kind="ExternalInput").ap()
    kw2AB_d = nc.dram_tensor("kw2AB", [128, 256], BF16, kind="ExternalInput").ap()
    fbias_d = nc.dram_tensor("fbias", [128, 4], F32, kind="ExternalInput").ap()
    out_d = nc.dram_tensor("out", [128, N], F32, kind="ExternalOutput").ap()
    g_dram = nc.dram_tensor("gscr", [BPC * 32768], BF16).ap()
    a_dram = nc.dram_tensor("ascr", [4096], BF16).ap()

    with tile.TileContext(nc) as tc:
        with (
            tc.tile_pool(name="const", bufs=1) as cpool,
            tc.tile_pool(name="work", bufs=1) as wpool,
            tc.tile_pool(name="ps_misc", bufs=1, space=bass.MemorySpace.PSUM) as pmisc,
            tc.tile_pool(name="ps_t1", bufs=2, space=bass.MemorySpace.PSUM) as pt1,
            tc.tile_pool(name="ps_t2", bufs=2, space=bass.MemorySpace.PSUM) as pt2,
            tc.tile_pool(name="ps_out", bufs=1, space=bass.MemorySpace.PSUM) as pout,
        ):
            blobA = cpool.tile([49, 1696], BF16, tag="blobA")
            blobB = cpool.tile([67, 1360], BF16, tag="blobB")
            kw2AB = cpool.tile([128, 256], BF16, tag="kw2AB")
            fbias = cpool.tile([128, 4], F32, tag="fbias")
            nc.sync.dma_start(blobA[:], blobA_d)
            nc.scalar.dma_start(blobB[:], blobB_d)
            nc.scalar.dma_start(kw2AB[:], kw2AB_d)
            nc.scalar.dma_start(fbias[:], fbias_d)

            c8T = blobA[0:8, A_C8T:A_C8T + 512]
            c4X = blobA[0:49, A_C4X:A_C4X + 32]
            L49 = blobA[0:49, A_L49:A_L49 + 128]
            xT = blobB[0:67, B_XT:B_XT + 512]
            dw1 = blobB[0:67, B_DW1:B_DW1 + 32]
            dw2 = blobB[0:32, B_DW2:B_DW2 + 16]
            dw3 = blobB[0:16, B_DW3:B_DW3 + 32]
            kw3p = blobB[0:32, B_KW3:B_KW3 + 256]
            kb3q = blobB[0:32, B_KB3Q:B_KB3Q + 512]
            db1 = fbias[0:32, 0:1]
            db2 = fbias[0:16, 1:2]
            db3 = fbias[0:32, 2:3]
            kb2t = fbias[0:128, 3:4]

            # ---- M2': a2bT[(b,chunk), (jl,h)] = a[chunk*16+jl,h]+kb1[h]
            a2bT_ps = pmisc.tile([32, 128], F32, tag="m")
            nc.tensor.matmul(a2bT_ps[:], c4X, L49)
            a2bT = wpool.tile([32, 128], BF16, tag="a2bT")
            nc.scalar.activation(a2bT[:], a2bT_ps[:], COPY)
            # bounce into blobA comp rows 4-7: dst[4+b, (c,jlh)] is
            # contiguous per b in (row-major) a_dram
            nc.sync.dma_start(a_dram, a2bT[:])
            nc.sync.dma_start(blobA[4:8, A_COMP:A_COMP + 1024],
                              a_dram.rearrange("(b x) -> b x", b=4))

            # ---- decode MLP -> fT [32, (b,i)] bf16 ----
            h1 = wpool.tile([32, BPC * N], BF16, tag="h1")
            h2 = wpool.tile([16, BPC * N], BF16, tag="h2")
            fT = wpool.tile([32, BPC * N], BF16, tag="fT")
            d1_ps = pmisc.tile([32, BPC * N], F32, tag="m")
            nc.tensor.matmul(d1_ps[:], dw1, xT)
            nc.scalar.activation(h1[:], d1_ps[:], RELU, bias=db1)
            d2_ps = pmisc.tile([16, BPC * N], F32, tag="m")
            nc.tensor.matmul(d2_ps[:], dw2, h1[:])
            nc.scalar.activation(h2[:], d2_ps[:], RELU, bias=db2)
            d3_ps = pmisc.tile([32, BPC * N], F32, tag="m")
            nc.tensor.matmul(d3_ps[:], dw3, h2[:])
            nc.scalar.activation(fT[:], d3_ps[:], RELU, bias=db3)

            # ---- t1: 8 K=8 matmuls + plain relu evacuations ----
            t1_sb = [wpool.tile([128, BPC * N], BF16, tag=f"t1_{c}",
                                name=f"t1sb{c}") for c in range(8)]
            for c in range(8):
                t1_ps = pt1.tile([128, BPC * N], F32, tag="t1ps",
                                 name=f"t1ps{c}")
                nc.tensor.matmul(
                    t1_ps[:],
                    blobA[0:8, A_COMP + c * 128:A_COMP + (c + 1) * 128],
                    c8T)
                if c % 2 == 0:
                    nc.vector.tensor_scalar(t1_sb[c][:], t1_ps[:], 0.0, None,
                                            MAX)
                else:
                    nc.scalar.activation(t1_sb[c][:], t1_ps[:], RELU)

            # ---- g: g_rm[j, (b,k,d)] -> DRAM bounce -> g_all[(jl,k),(b,T,d)]
            g_rm = wpool.tile([128, BPC * 256], BF16, tag="grm")
            g_all = wpool.tile([128, BPC * 256], BF16, tag="gall")
            for half in range(2):
                g_ps = pmisc.tile([128, 512], F32, tag="m", name=f"gps{half}")
                for bb in range(2):
                    b = half * 2 + bb
                    nc.tensor.matmul(g_ps[:, bb * 256:(bb + 1) * 256],
                                     fT[0:32, b * N:(b + 1) * N], kw3p)
                dst = g_rm[:, half * 512:(half + 1) * 512]
                if half == 0:
                    nc.vector.tensor_scalar(dst, g_ps[:], 0.0, None, ADD)
                else:
                    nc.scalar.activation(dst, g_ps[:], COPY)
                for bb in range(2):
                    b = half * 2 + bb
                    qe = nc.sync if bb == 0 else nc.scalar
                    qe.dma_start(g_dram[b * 32768:(b + 1) * 32768],
                                 g_rm[:, b * 256:(b + 1) * 256])
            for b in range(BPC):
                qe = nc.sync if b % 2 == 0 else nc.scalar
                dstB = g_all[:, b * 256:(b + 1) * 256].rearrange(
                    "p (c d) -> p c d", d=16)
                srcB = g_dram[b * 32768:(b + 1) * 32768].rearrange(
                    "(c jk d) -> jk c d", jk=128, d=16)
                qe.dma_start(dstB, srcB)

            # ---- bias2 in quadrant layout: [32b+d, 1] ----
            F_f32 = wpool.tile([32, BPC], F32, tag="Ff")
            F_sb = wpool.tile([32, BPC], BF16, tag="F")
            for b in range(BPC):
                nc.vector.tensor_reduce(F_f32[:, b:b + 1],
                                        fT[0:32, b * N:(b + 1) * N],
                                        mybir.AxisListType.X, ADD)
            nc.gpsimd.tensor_copy(F_sb[:], F_f32[:])
            b2_ps = pmisc.tile([128, 1], F32, tag="m")
            for b in range(BPC):
                nc.tensor.matmul(b2_ps[:], kb3q[:, b * 128:(b + 1) * 128],
                                 F_sb[0:32, b:b + 1],
                                 start=(b == 0), stop=(b == 3))
            b2q = wpool.tile([128, 1], F32, tag="b2q")
            nc.scalar.activation(b2q[:], b2_ps[:], COPY)

            # ---- t2: 16 matmuls N=512 into paired 2-bank PSUM tiles ----
            t2_sb = [wpool.tile([128, 2 * BPC * N], BF16, tag=f"t2_{p}",
                                name=f"t2sb{p}") for p in range(8)]
            for p in range(8):
                t2_ps = pt2.tile([128, 1024], F32, tag="t2ps",
                                 name=f"t2ps{p}")
                for s in range(2):
                    t = 2 * p + s
                    c, half = divmod(t, 2)
                    nc.tensor.matmul(t2_ps[:, s * 512:(s + 1) * 512],
                                     kw2AB[:, half * 128:(half + 1) * 128],
                                     t1_sb[c][:])
                if p % 2 == 0:
                    nc.vector.tensor_scalar(t2_sb[p][:], t2_ps[:], kb2t, 0.0,
                                            ADD, MAX)
                else:
                    nc.scalar.activation(t2_sb[p][:], t2_ps[:], RELU,
                                         bias=kb2t)

            # ---- final: 64 col-tiled matmuls, PSUM quadrants ----
            out_ps = pout.tile([128, N], F32, tag="o")
            for t in range(16):
                p, s = divmod(t, 2)
                for b in range(BPC):
                    nc.tensor.matmul(
                        out_ps[32 * b:32 * b + C2, :],
                        g_all[:, b * 256 + t * 16:b * 256 + (t + 1) * 16],
                        t2_sb[p][:, s * 512 + b * N:s * 512 + (b + 1) * N],
                        start=(t == 0), stop=(t == 15),
                        tile_position=(0, 32 * b))
            out_sb = wpool.tile([128, N], F32, tag="osb")
            nc.scalar.activation(out_sb[:], out_ps[:], RELU, bias=b2q[:])
            nc.sync.dma_start(out_d, out_sb[:])

    nc.compile()
    return nc


def _host_inputs(feature, coordinates_v, dw1, db1, dw2, db2, dw3, db3,
                 kw1, kb1, kw2, kb2, kw3, kb3):
    """Per-core input maps. Pure layout transforms, no FLOPs."""
    f32 = np.float32
    blobA0 = np.zeros((49, 1696), f32)
    # L49: jl-selector x kw1 rows + kb1 row
    for jl in range(16):
        for c in range(3):
            blobA0[c * 16 + jl, A_L49 + jl * 8:A_L49 + (jl + 1) * 8] = kw1[c]
    blobA0[48, A_L49:A_L49 + 128] = np.tile(kb1, 16)
    # comp rows 0-2: -kw1 replicated over jl (cols (c-chunk, jl, h))
    rep = np.tile((-kw1)[:, None, :], (1, 16, 1)).reshape(3, 128)
    blobA0[0:3, A_COMP:A_COMP + 1024] = np.tile(rep, (1, 8))

    blobB0 = np.zeros((67, 1360), f32)
    blobB0[0:67, B_DW1:B_DW1 + 32] = dw1
    blobB0[0:32, B_DW2:B_DW2 + 16] = dw2
    blobB0[0:16, B_DW3:B_DW3 + 32] = dw3
    blobB0[0:32, B_KW3:B_KW3 + 256] = (
        kw3.reshape(16, 32, 16).transpose(1, 0, 2).reshape(32, 256))
    kb3r = kb3.reshape(32, 16)
    for b in range(4):
        blobB0[0:32, B_KB3Q + b * 128 + 32 * b:
               B_KB3Q + b * 128 + 32 * b + 16] = kb3r

    kw2AB = np.zeros((128, 256), f32)
    for jl8 in range(8):
        kw2AB[jl8 * 8:(jl8 + 1) * 8, jl8 * 16:(jl8 + 1) * 16] = kw2
        kw2AB[(jl8 + 8) * 8:(jl8 + 9) * 8, 128 + jl8 * 16:128 + (jl8 + 1) * 16] = kw2

    fbias = np.zeros((128, 4), f32)
    fbias[0:32, 0] = db1
    fbias[0:16, 1] = db2
    fbias[0:32, 2] = db3
    fbias[:, 3] = np.tile(kb2, 8)

    in_maps = []
    for cix in range(NCORES):
        fe = feature[cix * BPC:(cix + 1) * BPC]          # [4, 64]
        co = coordinates_v[cix * BPC:(cix + 1) * BPC]    # [4, 128, 3]
        blobA = blobA0.copy()
        blobB = blobB0.copy()
        for b in range(BPC):
            blobA[0:3, A_C8T + b * N:A_C8T + (b + 1) * N] = co[b].T
            blobA[4 + b, A_C8T + b * N:A_C8T + (b + 1) * N] = 1.0
            blobB[0:64, B_XT + b * N:B_XT + (b + 1) * N] = fe[b][:, None]
            blobB[64:67, B_XT + b * N:B_XT + (b + 1) * N] = co[b].T
        blobA[3, A_C8T:A_C8T + 512] = 1.0
        # c4X[(c,jl'), (b,chunk)] = co[b, chunk*16+jl', c]; row 48 = 1
        cr = co.transpose(2, 0, 1).reshape(3, BPC, 8, 16)  # [c, b, chunk, jl]
        blobA[0:48, A_C4X:A_C4X + 32] = cr.transpose(0, 3, 1, 2).reshape(48, 32)
        blobA[48, A_C4X:A_C4X + 32] = 1.0
        in_maps.append({
            "blobA": blobA.astype(BF), "blobB": blobB.astype(BF),
            "kw2AB": kw2AB.astype(BF), "fbias": fbias})
    return in_maps


def kernel(**inputs):
    global _CACHED_NC
    if _CACHED_NC is None:
        _CACHED_NC = build_nc()
    nc = _CACHED_NC
    in_maps = _host_inputs(
        np.asarray(inputs["feature"]), np.asarray(inputs["coordinates_v"]),
        np.asarray(inputs["dw1"]), np.asarray(inputs["db1"]),
        np.asarray(inputs["dw2"]), np.asarray(inputs["db2"]),
        np.asarray(inputs["dw3"]), np.asarray(inputs["db3"]),
        np.asarray(inputs["kw1"]), np.asarray(inputs["kb1"]),
        np.asarray(inputs["kw2"]), np.asarray(inputs["kb2"]),
        np.asarray(inputs["kw3"]), np.asarray(inputs["kb3"]))
    res = run_bass_kernel_spmd(nc, in_maps, list(range(NCORES)))
    out = np.empty((B, N, C2), np.float32)
    for cix in range(NCORES):
        r = res.results[cix]["out"]      # [128, N] quadrants
        for b in range(BPC):
            out[cix * BPC + b] = r[32 * b:32 * b + C2, :].T
    return out
